# revision 16
# baseline (speedup 1.0000x reference)
"""Trainium2 Bass kernel for nn_Mlp_StaticRoutedLoRAExpert.

Computation (per token chunk with static expert e):
    h = gelu(x @ w1.T + bias1 + SCALE * (x @ a1[e].T) @ b1[e].T)
    y = h @ w2.T + bias2 + SCALE * (h @ a2[e].T) @ b2[e].T

Design:
  * LoRA folded into the dense weights on the host:
        W1_eff[e] = w1 + SCALE * b1[e] @ a1[e]   (same for W2_eff)
    so the device kernel is a plain per-chunk-expert MLP.
  * Data-parallel over batch: 4 batch rows per core on 8 cores.
  * Tokens are host-packed grouped by expert, so each core runs
    expert-contiguous T=512 tiles with a single weight switch, and each
    tile's x load / y store is ONE large contiguous DMA.
  * All matmul operands bf16 (fp32 PSUM accumulate), single fused pass:
    h stays in SBUF - no DRAM round trip for the hidden activations.
"""

import numpy as np
import ml_dtypes

BF16 = ml_dtypes.bfloat16

SCALE = 128.0 / 64.0
B, S, IN, HID, OUT, E, R = 32, 1280, 768, 3072, 768, 2, 64
NCORES = 8
BPC = B // NCORES          # batch rows per core
TPC = BPC * S              # tokens per core
P = 128
KI = IN // P               # 6
KH = HID // P              # 24
KO = OUT // P              # 6
MAX_T = 512                # PSUM bank / fp32 moving-operand limit

_nc_cache: dict = {}


def _segments(chunk_sizes, eids):
    """Packed-order segments (batch_row, seq_start, length, expert):
    chunks sorted by expert id (stable), each expanded over batch rows."""
    order = sorted(range(len(eids)), key=lambda i: (eids[i], i))
    segs = []
    for ci in order:
        s0 = int(sum(chunk_sizes[:ci]))
        for b in range(BPC):
            segs.append((b, s0, int(chunk_sizes[ci]), int(eids[ci])))
    return segs


def _plan_tiles(chunk_sizes, eids):
    """Per-core tiles over the packed token stream: (tok_off, T, expert)."""
    segs = _segments(chunk_sizes, eids)
    tiles = []
    toff = 0
    i = 0
    while i < len(segs):
        e = segs[i][3]
        run = 0
        while i < len(segs) and segs[i][3] == e:
            run += segs[i][2]
            i += 1
        off = 0
        while off < run:
            t = min(MAX_T, run - off)
            tiles.append((toff + off, t, e))
            off += t
        toff += run
    return tuple(tiles)


def _build(tiles, mode="full", internal_io=False, repeat=1,
           psh_bufs=4, split_w1=False):
    import concourse.bacc as bacc
    import concourse.mybir as mybir
    import concourse.tile as tile

    dt = mybir.dt
    f32 = dt.float32
    bf16 = dt.bfloat16
    AF = mybir.ActivationFunctionType

    nc = bacc.Bacc("TRN2", target_bir_lowering=False, num_devices=NCORES)

    tpc = sum(t for _, t, _ in tiles)
    kin = "Internal" if internal_io else "ExternalInput"
    kout = "Internal" if internal_io else "ExternalOutput"
    if mode == "xread":
        # timing probe: xp stays ExternalInput, everything else Internal;
        # body = L repeats of "DMA all of xp into SBUF".
        kin = "Internal"
        kout = "Internal"

    xp_d = nc.dram_tensor(
        "xp", [P, KI * tpc], bf16,
        kind="ExternalInput" if mode == "xread" else kin,
    )
    w1_d = [nc.dram_tensor(f"w1e{e}", [P, KI, HID], bf16, kind=kin)
            for e in range(E)]
    w2_d = [nc.dram_tensor(f"w2e{e}", [P, KH, OUT], bf16, kind=kin)
            for e in range(E)]
    b1v_d = nc.dram_tensor("bias1", [P, KH], f32, kind=kin)
    b2v_d = nc.dram_tensor("bias2", [P, KO], f32, kind=kin)
    yp_d = nc.dram_tensor("yp", [P, KO * tpc], f32, kind=kout)
    probe_d = None
    if internal_io:
        probe_d = nc.dram_tensor("probe", [1, P], f32, kind="ExternalOutput")

    do_dma = mode in ("full", "dma")
    do_mm = mode in ("full", "mm")

    # expert runs in tile order: (expert, [tile indices])
    runs = []
    for i, (_, _, e) in enumerate(tiles):
        if runs and runs[-1][0] == e:
            runs[-1][1].append(i)
        else:
            runs.append((e, [i]))

    with tile.TileContext(nc) as tc:
        with (
            tc.tile_pool(name="bias", bufs=1) as bias_pool,
            tc.tile_pool(name="w", bufs=3) as wpool,
            tc.tile_pool(name="xp", bufs=3) as xpool,
            tc.tile_pool(name="hp", bufs=40) as hpool,
            tc.tile_pool(name="yp", bufs=2) as ypool,
            tc.tile_pool(name="psh", bufs=psh_bufs, space="PSUM") as psh,
            tc.tile_pool(name="psy", bufs=2, space="PSUM") as psy,
        ):
            bias1_s = bias_pool.tile([P, KH], f32, name="bias1s", tag="b1")
            nc.sync.dma_start(bias1_s[:], b1v_d.ap())
            bias2_s = bias_pool.tile([P, KO], f32, name="bias2s", tag="b2")
            nc.sync.dma_start(bias2_s[:], b2v_d.ap())

            def load_w1(e, split=False):
                w = wpool.tile([P, KI, HID], bf16, name=f"w1s{e}", tag="w")
                if split:
                    for k in range(KI):
                        nc.sync.dma_start(w[:, k, :], w1_d[e][:, k, :])
                else:
                    nc.sync.dma_start(w[:], w1_d[e].ap())
                return w

            def load_w2(e):
                w = wpool.tile([P, KH, OUT], bf16, name=f"w2s{e}", tag="w")
                nc.sync.dma_start(w[:], w2_d[e].ap())
                return w

            # probe-mode fixed tiles (so every allocated tile has a writer)
            xc_fixed = None
            yc_fixed = None
            xcs_fixed = None
            if mode == "mm":
                xc_fixed = xpool.tile([P, KI * MAX_T], bf16, name="xcf", tag="xc")
                nc.vector.memset(xc_fixed[:], 0.0)
            if mode == "dma":
                yc_fixed = ypool.tile([P, KO * MAX_T], f32, name="ycf", tag="yc")
                nc.vector.memset(yc_fixed[:], 0.0)
            if mode in ("dma", "xread"):
                xcs_fixed = [
                    xpool.tile([P, KI * MAX_T], bf16, name=f"xcf{i}", tag="xc")
                    for i in range(3)
                ]

            def body():
                w1_cur = load_w1(runs[0][0], split=split_w1)
                w2_cur = load_w2(runs[0][0])
                nxt = {}
                for ri, (e, tlist) in enumerate(runs):
                    if ri > 0:
                        w1_cur, w2_cur = nxt["w1"], nxt["w2"]
                    next_e = runs[ri + 1][0] if ri + 1 < len(runs) else None
                    for j, ti in enumerate(tlist):
                        toff, T, _ = tiles[ti]
                        prefetch = next_e is not None and j == len(tlist) - 1
                        if prefetch:
                            nxt["w1"] = load_w1(next_e)
                        if mode == "mm":
                            xc = xc_fixed
                        elif mode == "dma":
                            xc = xcs_fixed[ti % 3]
                            nc.sync.dma_start(
                                xc[:, :KI * T],
                                xp_d[:, KI * toff:KI * toff + KI * T],
                            )
                        else:
                            xc = xpool.tile([P, KI * T], bf16, name="xc", tag="xc")
                            if do_dma:
                                nc.sync.dma_start(
                                    xc[:], xp_d[:, KI * toff:KI * toff + KI * T]
                                )
                        hcs = []
                        for m in range(KH):
                            hc = None
                            if do_mm:
                                hc = hpool.tile([P, T], bf16, name="hc", tag="hc")
                                h_ps = psh.tile([P, T], f32, name="hps", tag="h")
                                for k in range(KI):
                                    nc.tensor.matmul(
                                        h_ps[:],
                                        w1_cur[:, k, m * P:(m + 1) * P],
                                        xc[:, k * T:(k + 1) * T],
                                        start=(k == 0), stop=(k == KI - 1),
                                    )
                                nc.scalar.activation(
                                    hc[:], h_ps[:], AF.Gelu,
                                    bias=bias1_s[:, m:m + 1],
                                )
                            hcs.append(hc)
                        if prefetch:
                            nxt["w2"] = load_w2(next_e)
                        if mode == "dma":
                            yc = yc_fixed
                        elif mode == "full":
                            yc = ypool.tile([P, KO * T], f32, name="yc", tag="yc")
                        else:
                            yc = None
                        for o in range(KO):
                            if do_mm:
                                y_ps = psy.tile([P, T], f32, name="yps", tag="y")
                                for m in range(KH):
                                    nc.tensor.matmul(
                                        y_ps[:],
                                        w2_cur[:, m, o * P:(o + 1) * P],
                                        hcs[m][:],
                                        start=(m == 0), stop=(m == KH - 1),
                                    )
                                yv = (
                                    ypool.tile([P, T], f32, name="yv", tag="yc")
                                    if yc is None else yc[:, o * T:(o + 1) * T]
                                )
                                nc.scalar.activation(
                                    yv, y_ps[:],
                                    AF.Identity, bias=bias2_s[:, o:o + 1],
                                )
                        if do_dma:
                            nc.sync.dma_start(
                                yp_d[:, KO * toff:KO * toff + KO * T],
                                yc[:, :KO * T],
                            )

            def xread_body():
                for ti, (toff, T, _) in enumerate(tiles):
                    xc = xcs_fixed[ti % 3]
                    nc.sync.dma_start(
                        xc[:, :KI * T],
                        xp_d[:, KI * toff:KI * toff + KI * T],
                    )

            if mode == "xread":
                if repeat == 1:
                    xread_body()
                else:
                    with tc.For_i(0, repeat):
                        xread_body()
            elif mode == "empty":
                pass
            elif repeat == 1:
                body()
            else:
                with tc.For_i(0, repeat):
                    body()

        if probe_d is not None:
            nc.sync.dma_start(probe_d.ap(), yp_d[0:1, 0:P])
    nc.compile()
    return nc


def _get_nc(tiles):
    nc = _nc_cache.get(tiles)
    if nc is None:
        nc = _nc_cache[tiles] = _build(tiles)
    return nc


def _pack_weights(w1, bias1, a1, b1, w2, bias2, a2, b2):
    """Fold LoRA into dense weights and lay out for SBUF residency."""
    w1e = w1[None, :, :] + SCALE * np.matmul(b1, a1)    # [E, HID, IN]
    w2e = w2[None, :, :] + SCALE * np.matmul(b2, a2)    # [E, OUT, HID]
    out = {}
    for e in range(E):
        # [HID, IN] -> w1^T [IN, HID] -> [P, KI, HID]
        out[f"w1e{e}"] = np.ascontiguousarray(
            w1e[e].T.reshape(KI, P, HID).transpose(1, 0, 2)
        ).astype(BF16)
        out[f"w2e{e}"] = np.ascontiguousarray(
            w2e[e].T.reshape(KH, P, OUT).transpose(1, 0, 2)
        ).astype(BF16)
    out["bias1"] = np.ascontiguousarray(bias1.reshape(KH, P).T)
    out["bias2"] = np.ascontiguousarray(bias2.reshape(KO, P).T)
    return out


def _run(inputs, trace=False):
    from concourse.bass_utils import run_bass_kernel_spmd

    x = np.asarray(inputs["x"], dtype=np.float32)
    w1 = np.asarray(inputs["w1"], dtype=np.float32)
    bias1 = np.asarray(inputs["bias1"], dtype=np.float32)
    a1 = np.asarray(inputs["a1"], dtype=np.float32)
    b1 = np.asarray(inputs["b1"], dtype=np.float32)
    w2 = np.asarray(inputs["w2"], dtype=np.float32)
    bias2 = np.asarray(inputs["bias2"], dtype=np.float32)
    a2 = np.asarray(inputs["a2"], dtype=np.float32)
    b2 = np.asarray(inputs["b2"], dtype=np.float32)
    chunk_sizes = tuple(int(v) for v in np.asarray(inputs["chunk_sizes"]))
    eids = tuple(int(v) for v in np.asarray(inputs["expert_indices"]))
    assert sum(chunk_sizes) == S

    tiles = _plan_tiles(chunk_sizes, eids)
    segs = _segments(chunk_sizes, eids)
    nc = _get_nc(tiles)

    shared = _pack_weights(w1, bias1, a1, b1, w2, bias2, a2, b2)
    # packed token index within a core: gather x rows in expert-sorted order
    idx = np.concatenate(
        [b * S + s0 + np.arange(sz) for (b, s0, sz, _) in segs]
    )

    in_maps = []
    for c in range(NCORES):
        xc_tok = x[c * BPC:(c + 1) * BPC].reshape(TPC, IN)[idx].astype(BF16)
        xT = np.ascontiguousarray(xc_tok.T)            # [IN, TPC]
        blocks = [
            xT[:, toff:toff + T].reshape(KI, P, T)
            .transpose(1, 0, 2).reshape(P, KI * T)
            for (toff, T, _) in tiles
        ]
        m = dict(shared)
        m["xp"] = np.ascontiguousarray(np.concatenate(blocks, axis=1))
        in_maps.append(m)

    res = run_bass_kernel_spmd(
        nc, in_maps, core_ids=list(range(NCORES)), trace=trace
    )

    y = np.empty((B, S, OUT), np.float32)
    for c in range(NCORES):
        ypk = res.results[c]["yp"]                     # [P, KO*TPC]
        yT = np.empty((OUT, TPC), np.float32)
        for (toff, T, _) in tiles:
            yT[:, toff:toff + T] = (
                ypk[:, KO * toff:KO * toff + KO * T]
                .reshape(P, KO, T).transpose(1, 0, 2).reshape(OUT, T)
            )
        ycore = np.empty((TPC, OUT), np.float32)
        ycore[idx] = yT.T
        y[c * BPC:(c + 1) * BPC] = ycore.reshape(BPC, S, OUT)
    return y, res


def kernel(**inputs) -> np.ndarray:
    y, _ = _run(inputs, trace=False)
    return y
